# revision 7
# baseline (speedup 1.0000x reference)
"""Trainium2 Bass kernel for a 6-layer post-LN Transformer encoder.

Strategy (8 NeuronCores), v2:
  - Sequence-parallel: cores 0-3 own batch 0, cores 4-7 own batch 1; each core
    owns 512 tokens. Weights replicated (bf16), host-repacked so every weight
    DMA is one contiguous descriptor per partition.
  - Per layer: ONE AllGather for K^T and one for V (bf16) per 4-core group,
    issued right after the K / V projections so they hide under the remaining
    projections and the first head-pair's score tiles.
  - Attention is software-pipelined: scores/exp of head-pair j run on
    Tensor/Scalar while attn@V accumulation of head-pair j-1 runs interleaved
    on Tensor, with a 17-deep bf16 tile ring for the exp outputs. Softmax
    denominator rides as a 65th ones-column in V; per-pair normalization uses
    an in-place reciprocal + a K=1 broadcast matmul (no DRAM round trips).
  - Wo output projection is contraction-outer for its first 2 output tiles so
    it overlaps the attention tail; LayerNorm statistics interleave two PSUM
    chains with squares computed on the Scalar engine.
"""

import numpy as np
import ml_dtypes

L, D, H, FF = 6, 1024, 16, 4096
DK = D // H          # 64
B, S = 2, 2048
NCORES = 8
R = 4                # cores per batch group
T = S // R           # 512 local tokens per core
DC = D // 128        # 8
FC = FF // 128       # 32
KC = S // 128        # 16
EPS = 1e-5
BF16 = ml_dtypes.bfloat16

_CACHE = {}


def _build_nc():
    import contextlib
    import concourse.bacc as bacc
    import concourse.mybir as mybir
    import concourse.tile as tile
    import concourse.bass as bass
    from concourse.bass import ts, ds

    f32 = mybir.dt.float32
    bf16 = mybir.dt.bfloat16
    AF = mybir.ActivationFunctionType
    OP = mybir.AluOpType

    nc = bacc.Bacc(num_devices=NCORES)

    # ---- parameters -----------------------------------------------------
    x0T = nc.declare_dram_parameter("x0T", [D, T], f32, isOutput=False)
    maskb = nc.declare_dram_parameter("maskb", [128, KC], f32, isOutput=False)
    # weights host-repacked partition-major (one contiguous read / partition)
    wq_p = nc.declare_dram_parameter("wq_p", [L, 2, 128, DC, 512], bf16, isOutput=False)
    wk_p = nc.declare_dram_parameter("wk_p", [L, 2, 128, DC, 512], bf16, isOutput=False)
    wv_p = nc.declare_dram_parameter("wv_p", [L, 2, 128, DC, 512], bf16, isOutput=False)
    wo_p = nc.declare_dram_parameter("wo_p", [L, 2, 128, DC, 512], bf16, isOutput=False)
    w1_p = nc.declare_dram_parameter("w1_p", [L, 8, 128, DC, 512], bf16, isOutput=False)
    w2_p = nc.declare_dram_parameter("w2_p", [L, DC, 128, FC, 128], bf16, isOutput=False)
    bq = nc.declare_dram_parameter("bq", [L, D], f32, isOutput=False)
    bk = nc.declare_dram_parameter("bk", [L, D], f32, isOutput=False)
    bvb = nc.declare_dram_parameter("bvb", [L, D], bf16, isOutput=False)
    bo = nc.declare_dram_parameter("bo", [L, D], f32, isOutput=False)
    b1 = nc.declare_dram_parameter("b1", [L, FF], f32, isOutput=False)
    b2 = nc.declare_dram_parameter("b2", [L, D], f32, isOutput=False)
    g1 = nc.declare_dram_parameter("g1", [L, D], f32, isOutput=False)
    be1 = nc.declare_dram_parameter("be1", [L, D], f32, isOutput=False)
    g2 = nc.declare_dram_parameter("g2", [L, D], f32, isOutput=False)
    be2 = nc.declare_dram_parameter("be2", [L, D], f32, isOutput=False)
    outT = nc.declare_dram_parameter("outT", [D, T], f32, isOutput=True)

    groups = [[0, 1, 2, 3], [4, 5, 6, 7]]

    with tile.TileContext(nc) as tc:
        ctx = contextlib.ExitStack()
        singles = ctx.enter_context(tc.tile_pool(name="singles", bufs=1))
        params = ctx.enter_context(tc.tile_pool(name="params", bufs=2))
        wpool = ctx.enter_context(tc.tile_pool(name="wpool", bufs=2))
        stpool = ctx.enter_context(tc.tile_pool(name="stpool", bufs=2))
        kgpool = ctx.enter_context(tc.tile_pool(name="kgpool", bufs=2))
        bigpool = ctx.enter_context(tc.tile_pool(name="bigpool", bufs=16))
        xbfpool = ctx.enter_context(tc.tile_pool(name="xbfpool", bufs=2))
        aopool = ctx.enter_context(tc.tile_pool(name="aopool", bufs=1))
        tbpool = ctx.enter_context(tc.tile_pool(name="tbpool", bufs=1))
        tmp = ctx.enter_context(tc.tile_pool(name="tmp", bufs=2))
        small = ctx.enter_context(tc.tile_pool(name="small", bufs=1))
        dram = ctx.enter_context(tc.tile_pool(name="dram", bufs=2, space="DRAM"))
        pscore = ctx.enter_context(tc.tile_pool(name="pscore", bufs=2, space="PSUM"))
        pav = ctx.enter_context(tc.tile_pool(name="pav", bufs=2, space="PSUM"))
        pmisc = ctx.enter_context(tc.tile_pool(name="pmisc", bufs=2, space="PSUM"))

        # ---- constants + resident state --------------------------------
        xT = singles.tile([128, DC, T], f32, name="xT")
        nc.sync.dma_start(out=xT, in_=x0T[:, :].rearrange("(c p) t -> p c t", p=128))
        mb_sb = singles.tile([128, KC], f32, name="mb_sb")
        nc.sync.dma_start(out=mb_sb, in_=maskb[:, :])
        ones_col = singles.tile([128, 1], f32, name="ones_col")
        nc.vector.memset(ones_col, 1.0)
        ones_row = singles.tile([1, 128], f32, name="ones_row")
        nc.vector.memset(ones_row, 1.0)
        ones_row_bf = singles.tile([1, 128], bf16, name="ones_row_bf")
        nc.vector.memset(ones_row_bf, 1.0)
        eps_sb = singles.tile([1, 1], f32, name="eps_sb")
        nc.vector.memset(eps_sb, EPS)
        ones_pp = singles.tile([128, 128], bf16, name="ones_pp")
        nc.vector.memset(ones_pp, 1.0)
        qT = singles.tile([128, DC, T], bf16, name="qT")
        vg = singles.tile([128, KC, H, 65], bf16, name="vg")
        nc.vector.memset(vg[:, :, :, 64:65], 1.0)
        an_bf = singles.tile([128, DC, T], bf16, name="an_bf")

        xbf_cur = xbfpool.tile([128, DC, T], bf16, tag="xbf")
        for c in range(DC):
            nc.vector.tensor_copy(xbf_cur[:, c, :], xT[:, c, :])

        def layernorm_inplace(g_sb, be_sb, xbf_out):
            """x = LN(x) in place; also writes bf16 copy into xbf_out."""
            ps_sum = pmisc.tile([128, 512], f32, tag="pmisc")
            for c in range(DC):
                nc.tensor.matmul(ps_sum[0:1, :], ones_col, xT[:, c, :],
                                 start=(c == 0), stop=(c == DC - 1))
            ps_sq = pmisc.tile([128, 512], f32, tag="pmisc")
            for c in range(DC):
                sq = tmp.tile([128, 512], f32, tag="tmp")
                nc.scalar.square(sq, xT[:, c, :])
                nc.tensor.matmul(ps_sq[0:1, :], ones_col, sq,
                                 start=(c == 0), stop=(c == DC - 1))
            mr = small.tile([1, 1024], f32, tag="mr")
            t2 = small.tile([1, 1024], f32, tag="t2")
            e2 = t2[:, 0:512]
            msq = t2[:, 512:1024]
            nc.scalar.mul(mr[:, 0:512], ps_sum[0:1, :], 1.0 / D)
            nc.scalar.mul(e2, ps_sq[0:1, :], 1.0 / D)
            nc.vector.tensor_mul(msq, mr[:, 0:512], mr[:, 0:512])
            nc.vector.tensor_tensor(e2, e2, msq, OP.subtract)
            lnv = t2[:, 512:1024]
            nc.scalar.activation(lnv, e2, AF.Ln, bias=eps_sb)
            nc.scalar.activation(mr[:, 512:1024], lnv, AF.Exp, scale=-0.5)
            bc = pscore.tile([128, 1024], f32, tag="ps")
            nc.tensor.matmul(bc[:, 0:512], ones_row, mr[:, 0:512],
                             start=True, stop=True)
            nc.tensor.matmul(bc[:, 512:1024], ones_row, mr[:, 512:1024],
                             start=True, stop=True)
            for c in range(DC):
                t1 = tmp.tile([128, 512], f32, tag="tmp")
                nc.vector.tensor_tensor(t1, xT[:, c, :], bc[:, 0:512], OP.subtract)
                nc.vector.tensor_tensor(t1, t1, bc[:, 512:1024], OP.mult)
                nc.scalar.activation(xT[:, c, :], t1, AF.Identity,
                                     bias=be_sb[:, c:c + 1], scale=g_sb[:, c:c + 1])
                nc.vector.tensor_copy(xbf_out[:, c, :], xT[:, c, :])

        for l in range(L):
            # ---- per-layer params --------------------------------------
            pp = params.tile([128, 8, DC], f32, tag="pcol")
            for i_, t_src in enumerate([bq, bk, bo, b2, g1, be1, g2, be2]):
                nc.sync.dma_start(out=pp[:, i_, :],
                                  in_=t_src[l].rearrange("(c p) -> p c", p=128))
            bq_sb, bk_sb, bo_sb, b2_sb = pp[:, 0], pp[:, 1], pp[:, 2], pp[:, 3]
            g1_sb, be1_sb, g2_sb, be2_sb = pp[:, 4], pp[:, 5], pp[:, 6], pp[:, 7]
            b1_sb = params.tile([128, FC], f32, tag="pc32")
            nc.sync.dma_start(out=b1_sb, in_=b1[l].rearrange("(c p) -> p c", p=128))
            bv_row = params.tile([1, D], bf16, tag="bv_row")
            nc.sync.dma_start(out=bv_row, in_=bvb[l][None, :])

            xbf = xbf_cur

            # ---- K projection + AllGather ------------------------------
            agk_in = dram.tile([D, T], bf16, tag="agk_in")
            agk_out = dram.tile([R * D, T], bf16, tag="agk_out")
            agk_view = agk_in[:, :].rearrange("(c p) t -> p c t", p=128)
            for half in range(2):
                wk_h = wpool.tile([128, DC, 512], bf16, tag="wqkv")
                nc.sync.dma_start(out=wk_h, in_=wk_p[l, half])
                for mcb in range(4):
                    mc = half * 4 + mcb
                    ps = pmisc.tile([128, 512], f32, tag="pmisc")
                    for c in range(DC):
                        nc.tensor.matmul(ps, wk_h[:, c, ts(mcb, 128)], xbf[:, c, :],
                                         start=(c == 0), stop=(c == DC - 1))
                    st = stpool.tile([128, 512], bf16, tag="st")
                    nc.vector.tensor_scalar(st, ps, bk_sb[:, mc:mc + 1], None, OP.add)
                    nc.gpsimd.dma_start(out=agk_view[:, mc, :], in_=st)
            nc.gpsimd.collective_compute(
                "AllGather", OP.bypass, replica_groups=groups,
                ins=[agk_in.opt()], outs=[agk_out.opt()])

            # ---- V projection + AllGather ------------------------------
            agv_in = dram.tile([2 * T, 512], bf16, tag="agv_in")
            agv_out = dram.tile([R * 2 * T, 512], bf16, tag="agv_out")
            agv_iview = agv_in[:, :].rearrange("(c p two) f -> p c two f",
                                               p=128, two=2)
            for half in range(2):
                wv_h = wpool.tile([128, DC, 512], bf16, tag="wqkv")
                nc.sync.dma_start(out=wv_h, in_=wv_p[l, half])
                for t_ in range(4):
                    ps = pmisc.tile([128, 512], f32, tag="pmisc")
                    for c in range(DC):
                        nc.tensor.matmul(ps, xbf[:, c, ts(t_, 128)],
                                         wv_h[:, c, :],
                                         start=(c == 0), stop=False)
                    nc.tensor.matmul(ps, ones_row_bf, bv_row[:, ds(half * 512, 512)],
                                     start=False, stop=True)
                    st = stpool.tile([128, 512], bf16, tag="st")
                    nc.vector.tensor_copy(st, ps)
                    nc.gpsimd.dma_start(out=agv_iview[:, t_, half, :], in_=st)
            nc.gpsimd.collective_compute(
                "AllGather", OP.bypass, replica_groups=groups,
                ins=[agv_in.opt()], outs=[agv_out.opt()])

            # ---- Q projection ------------------------------------------
            for half in range(2):
                wq_h = wpool.tile([128, DC, 512], bf16, tag="wqkv")
                nc.sync.dma_start(out=wq_h, in_=wq_p[l, half])
                for mcb in range(4):
                    mc = half * 4 + mcb
                    ps = pmisc.tile([128, 512], f32, tag="pmisc")
                    for c in range(DC):
                        nc.tensor.matmul(ps, wq_h[:, c, ts(mcb, 128)], xbf[:, c, :],
                                         start=(c == 0), stop=(c == DC - 1))
                    nc.vector.tensor_scalar(qT[:, mc, :], ps,
                                            bq_sb[:, mc:mc + 1], None, OP.add)

            # ---- Wo prefetch -------------------------------------------
            wo_a = wpool.tile([128, DC, 512], bf16, tag="wo")
            nc.sync.dma_start(out=wo_a, in_=wo_p[l, 0])
            wo_b = wpool.tile([128, DC, 512], bf16, tag="wo")
            nc.sync.dma_start(out=wo_b, in_=wo_p[l, 1])

            # ---- gathered V -> SBUF (after AG_V) -----------------------
            agv_view = agv_out[:, :].rearrange(
                "(r c p two) f -> p r c (two f)", p=128, two=2, c=4)
            for kc in range(KC):
                r, t_ = kc // 4, kc % 4
                nc.sync.dma_start(
                    out=vg[:, kc, :, 0:64],
                    in_=agv_view[:, r, t_, :].rearrange("p (h d) -> p h d", d=64))

            # ---- attention ---------------------------------------------
            def emit_kg(jj):
                kgj = kgpool.tile([128, R, T], bf16, tag="kg")
                for r in range(R):
                    nc.sync.dma_start(
                        out=kgj[:, r, :],
                        in_=agk_out[ds(r * D + jj * 128, 128), :])
                return kgj

            def emit_chain_mm(jj, kc, last):
                pA, pB, atj = pav_state[jj]
                at_t = atj[kc]
                nc.tensor.matmul(pA[0:65, :], vg[:, kc, 2 * jj, :], at_t[:, 0:512],
                                 start=(kc == 0), stop=last)
                nc.tensor.matmul(pB[0:65, :], vg[:, kc, 2 * jj + 1, :],
                                 at_t[:, 512:1024],
                                 start=(kc == 0), stop=last)

            def emit_tail(jj):
                pA, pB, _ = pav_state[jj]
                ao = aopool.tile([128, 2, 512], bf16, tag="ao")
                nc.vector.tensor_copy(ao[0:64, 0, :], pA[0:64, :])
                nc.vector.tensor_copy(ao[0:64, 1, :], pB[0:64, :])
                with nc.allow_low_precision("softmax denom reciprocal in bf16"):
                    nc.vector.reciprocal(ao[64:65, 0, :], pA[64:65, :])
                    nc.vector.reciprocal(ao[64:65, 1, :], pB[64:65, :])
                bc = pscore.tile([128, 1024], f32, tag="ps")
                nc.tensor.matmul(bc[:, 0:512], ones_pp[64:65, :], ao[64:65, 0, :],
                                 start=True, stop=True)
                nc.tensor.matmul(bc[:, 512:1024], ones_pp[64:65, :], ao[64:65, 1, :],
                                 start=True, stop=True)
                tb = tbpool.tile([128, 512], bf16, tag="tb")
                nc.sync.dma_start(out=tb[ds(64, 64), :], in_=ao[0:64, 1, :])
                nc.vector.tensor_tensor(an_bf[0:64, jj, :], ao[0:64, 0, :],
                                        bc[0:64, 0:512], OP.mult)
                nc.vector.tensor_tensor(an_bf[ds(64, 64), jj, :], tb[ds(64, 64), :],
                                        bc[ds(64, 64), 512:1024], OP.mult)

            psW = [None, None]

            def emit_wo_g0(c):
                for mcb in range(2):
                    if c == 0:
                        psW[mcb] = pmisc.tile([128, 512], f32, tag="pmisc", name="psW")
                    nc.tensor.matmul(psW[mcb], wo_a[:, c, ts(mcb, 128)],
                                     an_bf[:, c, :],
                                     start=(c == 0), stop=(c == DC - 1))

            pav_state = {}
            at_list = []
            kgj_cur = emit_kg(0)
            for j in range(DC):
                kgj = kgj_cur
                if j + 1 < DC:
                    kgj_cur = emit_kg(j + 1)
                if j >= 1:
                    pav_state[j - 1] = (
                        pav.tile([128, 512], f32, tag="pav", name="pavA"),
                        pav.tile([128, 512], f32, tag="pav", name="pavB"),
                        at_list)
                at_list = []
                for kc in range(KC):
                    r, t_ = kc // 4, kc % 4
                    pss = pscore.tile([128, 1024], f32, tag="ps")
                    nc.tensor.matmul(pss[:, 0:512], kgj[0:64, r, ts(t_, 128)],
                                     qT[0:64, j, :], start=True, stop=True,
                                     tile_position=(0, 0))
                    nc.tensor.matmul(pss[:, 512:1024], kgj[ds(64, 64), r, ts(t_, 128)],
                                     qT[ds(64, 64), j, :], start=True, stop=True,
                                     tile_position=(64, 0))
                    at_t = bigpool.tile([128, 1024], bf16, tag="big")
                    nc.scalar.activation(at_t, pss, AF.Exp, scale=1.0 / 32.0,
                                         bias=mb_sb[:, kc:kc + 1])
                    at_list.append(at_t)
                    if j >= 1:
                        emit_chain_mm(j - 1, kc, last=(kc == KC - 1))
                if j >= 1:
                    emit_tail(j - 1)
                if j >= 2:
                    emit_wo_g0(j - 2)
            # flush: chains + tail for j=7, Wo g0 remainder
            pav_state[DC - 1] = (pav.tile([128, 512], f32, tag="pav", name="pavA"),
                                 pav.tile([128, 512], f32, tag="pav", name="pavB"),
                                 at_list)
            for kc in range(KC):
                emit_chain_mm(DC - 1, kc, last=(kc == KC - 1))
            emit_tail(DC - 1)
            emit_wo_g0(DC - 2)
            emit_wo_g0(DC - 1)
            for mcb in range(2):
                nc.vector.scalar_tensor_tensor(xT[:, mcb, :], psW[mcb],
                                               bo_sb[:, mcb:mcb + 1], xT[:, mcb, :],
                                               OP.add, OP.add)
            for g in range(1, 4):
                psW2 = [pmisc.tile([128, 512], f32, tag="pmisc", name="psW2") for _ in range(2)]
                for c in range(DC):
                    for i_ in range(2):
                        mc = 2 * g + i_
                        wo_h = wo_a if mc < 4 else wo_b
                        nc.tensor.matmul(psW2[i_], wo_h[:, c, ts(mc % 4, 128)],
                                         an_bf[:, c, :],
                                         start=(c == 0), stop=(c == DC - 1))
                for i_ in range(2):
                    mc = 2 * g + i_
                    nc.vector.scalar_tensor_tensor(xT[:, mc, :], psW2[i_],
                                                   bo_sb[:, mc:mc + 1], xT[:, mc, :],
                                                   OP.add, OP.add)

            # ---- LN1 ----------------------------------------------------
            xbf2 = xbfpool.tile([128, DC, T], bf16, tag="xbf")
            layernorm_inplace(g1_sb, be1_sb, xbf2)

            # ---- FFN ----------------------------------------------------
            ffts = []
            for b4 in range(8):
                w1_h = wpool.tile([128, DC, 512], bf16, tag="w1")
                nc.sync.dma_start(out=w1_h, in_=w1_p[l, b4])
                for mcb in range(4):
                    mc = b4 * 4 + mcb
                    if mc % 2 == 0:
                        ffts.append(bigpool.tile([128, 2, 512], bf16, tag="big", name="fft"))
                    ps = pmisc.tile([128, 512], f32, tag="pmisc")
                    for c in range(DC):
                        nc.tensor.matmul(ps, w1_h[:, c, ts(mcb, 128)], xbf2[:, c, :],
                                         start=(c == 0), stop=(c == DC - 1))
                    nc.vector.tensor_scalar(ffts[mc // 2][:, mc % 2, :], ps,
                                            b1_sb[:, mc:mc + 1], 0.0,
                                            OP.add, OP.max)
            for mc in range(DC):
                w2_sb = wpool.tile([128, FC, 128], bf16, tag="w2")
                nc.sync.dma_start(out=w2_sb, in_=w2_p[l, mc])
                ps = pmisc.tile([128, 512], f32, tag="pmisc")
                for fc in range(FC):
                    nc.tensor.matmul(ps, w2_sb[:, fc, :], ffts[fc // 2][:, fc % 2, :],
                                     start=(fc == 0), stop=(fc == FC - 1))
                nc.vector.scalar_tensor_tensor(xT[:, mc, :], ps,
                                               b2_sb[:, mc:mc + 1], xT[:, mc, :],
                                               OP.add, OP.add)

            # ---- LN2 ----------------------------------------------------
            xbf_cur = xbfpool.tile([128, DC, T], bf16, tag="xbf")
            layernorm_inplace(g2_sb, be2_sb, xbf_cur)

        # ---- output ----------------------------------------------------
        nc.sync.dma_start(out=outT[:, :].rearrange("(c p) t -> p c t", p=128), in_=xT)
        ctx.close()

    nc.compile()
    return nc


def _prepare_host(inputs):
    src = np.asarray(inputs["src"]).astype(np.int64)
    emb = np.asarray(inputs["emb"], dtype=np.float32)
    x = emb[src]                                    # [B, S, D] f32
    pos = np.arange(B, dtype=np.float32)[:, None]
    div = np.exp(np.arange(0, D, 2, dtype=np.float32) * (-np.log(10000.0) / D))
    pe = np.zeros((B, D), np.float32)
    pe[:, 0::2] = np.sin(pos / div)
    pe[:, 1::2] = np.cos(pos / div)
    x = x + pe[:, None, :]

    mask = np.asarray(inputs["src_mask"]).reshape(B, S)
    mbias = np.where(mask != 0, 0.0, -1e9).astype(np.float32)   # [B, S]

    f = np.float32

    def pack_proj(w):  # [L, D, M] -> [L, 2, 128, DC, 512] (m-halves)
        w = np.asarray(w, f).astype(BF16)
        return np.ascontiguousarray(
            w.reshape(L, DC, 128, 2, 512).transpose(0, 3, 2, 1, 4))

    w1 = np.asarray(inputs["W1"], f).astype(BF16)
    w2 = np.asarray(inputs["W2"], f).astype(BF16)
    shared = {
        "wq_p": pack_proj(inputs["Wq"]),
        "wk_p": pack_proj(inputs["Wk"]),
        "wv_p": pack_proj(inputs["Wv"]),
        "wo_p": pack_proj(inputs["Wo"]),
        "w1_p": np.ascontiguousarray(
            w1.reshape(L, DC, 128, 8, 512).transpose(0, 3, 2, 1, 4)),
        "w2_p": np.ascontiguousarray(
            w2.reshape(L, FC, 128, DC, 128).transpose(0, 3, 2, 1, 4)),
        "bq": np.ascontiguousarray(np.asarray(inputs["bq"], f)),
        "bk": np.ascontiguousarray(np.asarray(inputs["bk"], f)),
        "bvb": np.ascontiguousarray(np.asarray(inputs["bv"], f).astype(BF16)),
        "bo": np.ascontiguousarray(np.asarray(inputs["bo"], f)),
        "b1": np.ascontiguousarray(np.asarray(inputs["b1"], f)),
        "b2": np.ascontiguousarray(np.asarray(inputs["b2"], f)),
        "g1": np.ascontiguousarray(np.asarray(inputs["g1"], f)),
        "be1": np.ascontiguousarray(np.asarray(inputs["be1"], f)),
        "g2": np.ascontiguousarray(np.asarray(inputs["g2"], f)),
        "be2": np.ascontiguousarray(np.asarray(inputs["be2"], f)),
    }
    in_maps = []
    for i in range(NCORES):
        b = i // R
        t0 = (i % R) * T
        m = dict(shared)
        m["x0T"] = np.ascontiguousarray(x[b, t0:t0 + T, :].T.astype(np.float32))
        m["maskb"] = np.ascontiguousarray(mbias[b].reshape(KC, 128).T)
        in_maps.append(m)
    return in_maps


def _run(in_maps, trace=False):
    from concourse.bass_utils import run_bass_kernel_spmd
    if "nc" not in _CACHE:
        _CACHE["nc"] = _build_nc()
    nc = _CACHE["nc"]
    res = run_bass_kernel_spmd(nc, in_maps, core_ids=list(range(NCORES)),
                               trace=trace)
    outs = res.results
    y = np.zeros((B, S, D), np.float32)
    for i in range(NCORES):
        b = i // R
        t0 = (i % R) * T
        y[b, t0:t0 + T, :] = outs[i]["outT"].T
    return y, res


def kernel(**inputs) -> np.ndarray:
    in_maps = _prepare_host(inputs)
    y, _ = _run(in_maps, trace=False)
    return y


def kernel_traced(**inputs):
    """Same as kernel() but returns (output, BassKernelResults with profile)."""
    in_maps = _prepare_host(inputs)
    return _run(in_maps, trace=True)
